# revision 1
# baseline (speedup 1.0000x reference)
"""Trainium2 Bass kernel for a DP-GAT layer (dense masked attention).

Computes, for x:[B,N,D], A_shape:[N,N] (0/1 adjacency), q,k,v:[D,D]:
    Q = x@q ; K = x@k
    S = Q @ K^T / sqrt(D)
    W = exp(8*tanh(S/8)) * A_shape
    out = (W / W.sum(-1, keepdims=True)) @ x @ v

Sharding: rows of N split across 8 NeuronCores (1024 rows each), SPMD,
no collectives. Each core streams its row-block of the mask, computes
scores in a flash-attention-style fused loop, and writes its row-block
of the output. Host scatters inputs / gathers outputs.

Numerics: q,k are split on the host into fp16 hi+lo pairs; K^T and Q^T
are computed as two-pass fp16 matmuls with fp32 PSUM accumulation and
stored as fp16. fp16 score operands keep the final output within ~2e-3
relative of the fp32 reference (fp16 matmuls run at full PE rate, and
the exp(8*tanh) amplification of coarser dtypes is unacceptable).

Device-side flow (per core, per batch):
    KT  = k^T @ x^T  (fp16 2-pass)   [D, N]
    QT  = q^T @ xrows^T (fp16 2-pass)[D, RB]
    xv  = x @ v (+ ones col)         [N, D+1] fp16
    per i-chunk of 512 query rows:
      per group of 4 key-tiles (512 keys):
        S^T  = KT_tile^T @ QT_chunk      -> PSUM [128, 4, 512] fp32
        u    = tanh(S^T / (8*sqrt(D)))   -> SBUF fp32  (ScalarE, scale fused)
        w    = exp(8*u)                  -> SBUF fp16  (ScalarE, scale fused)
        p    = w * maskT_tile            -> SBUF fp16  (VectorE)
        acc[i,0:129] += p_slice^T @ xv   -> PSUM       (fp16 matmuls; col 128
                                                        = rowsum via ones col)
      out = acc[:, :128] * (1/acc[:, 128])  -> DMA to DRAM

The per-batch prep (KT/QT/xv) is software-pipelined: prep chunk g of
batch b+1 is emitted between groups of batch b's second i-chunk (and
batch 0's prep between its own first-i-chunk groups, which is legal
because chunk g produces exactly the kt columns / xv tiles group g
consumes), so the PE never runs a long prep burst while ScalarE idles.

PSUM bank budget (8 banks of 2KB): score group 4 + PV accumulator 2 +
prep 2. PE matmuls with start=True clear their entire output PSUM bank,
so the two acc slots sharing a bank are zeroed by one full-bank dummy
matmul and all PV matmuls accumulate with start=False.
"""

import math
import sys
from contextlib import ExitStack

import numpy as np

try:
    import concourse.bass as bass  # noqa: F401
except ImportError:  # pragma: no cover
    sys.path.insert(0, "/opt/trn_rl_repo")
    import concourse.bass as bass  # noqa: F401

import concourse.mybir as mybir
import concourse.tile as tile
from concourse import bacc
from concourse.bass_utils import run_bass_kernel_spmd

F32 = mybir.dt.float32
F16 = mybir.dt.float16

B, N, D = 4, 8192, 128
NCORES = 8
RB = N // NCORES  # query rows per core

IC = 512          # query-row chunk (free dim of score matmuls)
NIC = RB // IC    # i-chunks per core
JG = 4            # key 128-tiles per score group
NJT = N // 128    # key tiles total
NG = NJT // JG    # groups per i-chunk
CH = JG * 128     # xt prep chunk width (chunk g produces what group g consumes)


def build_program():
    nc = bacc.Bacc("TRN2", target_bir_lowering=False, debug=False)

    xt = nc.dram_tensor("xt", [B, D, N], F16, kind="ExternalInput").ap()
    xqt = nc.dram_tensor("xqt", [B, D, RB], F16, kind="ExternalInput").ap()
    maskT = nc.dram_tensor("maskT", [N, RB], F16, kind="ExternalInput").ap()
    qh_d = nc.dram_tensor("q_hi", [D, D], F16, kind="ExternalInput").ap()
    ql_d = nc.dram_tensor("q_lo", [D, D], F16, kind="ExternalInput").ap()
    kh_d = nc.dram_tensor("k_hi", [D, D], F16, kind="ExternalInput").ap()
    kl_d = nc.dram_tensor("k_lo", [D, D], F16, kind="ExternalInput").ap()
    v_d = nc.dram_tensor("v", [D, D], F16, kind="ExternalInput").ap()
    out_d = nc.dram_tensor("out", [B, RB, D], F32, kind="ExternalOutput").ap()

    # [128, key-tile, query-col] view of the transposed mask block
    maskT_r = maskT.rearrange("(t p) i -> p t i", p=128)

    tanh_scale = 1.0 / (8.0 * math.sqrt(float(D)))

    with tile.TileContext(nc) as tc, ExitStack() as ctx:
        consts = ctx.enter_context(tc.tile_pool(name="consts", bufs=1))
        kt_pool = ctx.enter_context(tc.tile_pool(name="kt", bufs=2))
        qt_pool = ctx.enter_context(tc.tile_pool(name="qt", bufs=2))
        xv_pool = ctx.enter_context(tc.tile_pool(name="xv", bufs=2))
        xc_pool = ctx.enter_context(tc.tile_pool(name="xc", bufs=3))
        m_pool = ctx.enter_context(tc.tile_pool(name="m", bufs=3))
        u_pool = ctx.enter_context(tc.tile_pool(name="u", bufs=2))
        w_pool = ctx.enter_context(tc.tile_pool(name="w", bufs=2))
        p_pool = ctx.enter_context(tc.tile_pool(name="p", bufs=2))
        ob_pool = ctx.enter_context(tc.tile_pool(name="ob", bufs=4))
        rs_pool = ctx.enter_context(tc.tile_pool(name="rs", bufs=4))
        prep_ps = ctx.enter_context(tc.tile_pool(name="prep_ps", bufs=2, space="PSUM"))
        st_ps = ctx.enter_context(tc.tile_pool(name="st_ps", bufs=1, space="PSUM"))
        acc_ps = ctx.enter_context(tc.tile_pool(name="acc_ps", bufs=1, space="PSUM"))

        zeros = consts.tile([128, 512], F16)
        nc.vector.memset(zeros[:], 0.0)
        qh_sb = consts.tile([D, D], F16)
        nc.sync.dma_start(qh_sb[:], qh_d[:])
        ql_sb = consts.tile([D, D], F16)
        nc.sync.dma_start(ql_sb[:], ql_d[:])
        kh_sb = consts.tile([D, D], F16)
        nc.sync.dma_start(kh_sb[:], kh_d[:])
        kl_sb = consts.tile([D, D], F16)
        nc.sync.dma_start(kl_sb[:], kl_d[:])
        v_sb = consts.tile([D, D], F16)
        nc.sync.dma_start(v_sb[:], v_d[:])

        tiles = {}  # b -> (kt, qt, xv)

        def prep_head(b):
            """Allocate batch-b tiles; compute QT; set xv ones column."""
            kt = kt_pool.tile([128, N], F16)
            qt = qt_pool.tile([128, RB], F16)
            xv = xv_pool.tile([128, NJT, 130], F16)
            tiles[b] = (kt, qt, xv)
            nc.vector.memset(xv[:, :, 128:129], 1.0)
            xq = qt_pool.tile([128, RB], F16, tag="xq")
            nc.sync.dma_start(xq[:], xqt[b])
            qch = min(CH, RB)
            for c in range(RB // qch):
                pq = prep_ps.tile([128, qch], F32, tag="prep")
                nc.tensor.matmul(
                    pq[:], qh_sb[:], xq[:, c * qch : (c + 1) * qch],
                    start=True, stop=False,
                )
                nc.tensor.matmul(
                    pq[:], ql_sb[:], xq[:, c * qch : (c + 1) * qch],
                    start=False, stop=True,
                )
                nc.vector.tensor_copy(qt[:, c * qch : (c + 1) * qch], pq[:])

        def prep_chunk(b, c):
            """Compute kt columns and xv tiles for chunk c of batch b."""
            kt, _, xv = tiles[b]
            xc = xc_pool.tile([128, CH], F16)
            nc.sync.dma_start(xc[:], xt[b][:, c * CH : (c + 1) * CH])
            pk = prep_ps.tile([128, CH], F32, tag="prep")
            nc.tensor.matmul(pk[:], kh_sb[:], xc[:], start=True, stop=False)
            nc.tensor.matmul(pk[:], kl_sb[:], xc[:], start=False, stop=True)
            nc.vector.tensor_copy(kt[:, c * CH : (c + 1) * CH], pk[:])
            for s in range(CH // 128):
                pxv = prep_ps.tile([128, 128], F32, tag="prep")
                nc.tensor.matmul(
                    pxv[:], xc[:, s * 128 : (s + 1) * 128], v_sb[:],
                    start=True, stop=True,
                )
                nc.vector.tensor_copy(xv[:, c * (CH // 128) + s, 0:128], pxv[:])

        def zero_acc(acc):
            # PE start=True clears the WHOLE PSUM bank, so the two acc
            # slots sharing a bank are zeroed by one full-bank dummy
            # matmul; all real PV matmuls accumulate with start=False.
            for hb in range(2):
                nc.tensor.matmul(
                    acc[:, hb * 512 : (hb + 1) * 512],
                    zeros[:, 0:128], zeros[:],
                    start=True, stop=False, skip_group_check=True,
                )

        def group(b, ic, g, acc):
            kt, qt, xv = tiles[b]
            stp = st_ps.tile([128, JG, IC], F32)
            for j in range(JG):
                nc.tensor.matmul(
                    stp[:, j],
                    kt[:, (g * JG + j) * 128 : (g * JG + j + 1) * 128],
                    qt[:, ic * IC : (ic + 1) * IC],
                    start=True, stop=True,
                )
            if g == 0:
                # placed after the first score matmuls so the PE can issue
                # them while the previous i-chunk's normalize drains
                zero_acc(acc)
            u = u_pool.tile([128, JG, IC], F32)
            nc.scalar.activation(
                u[:], stp[:], mybir.ActivationFunctionType.Tanh, scale=tanh_scale
            )
            w = w_pool.tile([128, JG, IC], F16)
            nc.scalar.activation(
                w[:], u[:], mybir.ActivationFunctionType.Exp, scale=8.0
            )
            m = m_pool.tile([128, JG, IC], F16)
            nc.sync.dma_start(
                m[:], maskT_r[:, g * JG : (g + 1) * JG, ic * IC : (ic + 1) * IC]
            )
            p = p_pool.tile([128, JG, IC], F16)
            nc.vector.tensor_mul(p[:], w[:], m[:])
            for j in range(JG):
                for s in range(IC // 128):
                    nc.tensor.matmul(
                        acc[:, s * 256 : s * 256 + 129],
                        p[:, j, s * 128 : (s + 1) * 128],
                        xv[:, g * JG + j, 0:129],
                        start=False,
                        stop=(g == NG - 1 and j == JG - 1),
                        skip_group_check=True,
                    )

        prep_head(0)
        for b in range(B):
            for ic in range(NIC):
                if ic == NIC - 1 and b + 1 < B:
                    prep_head(b + 1)
                acc = acc_ps.tile([128, 1024], F32)
                if b == 0 and ic == 0:
                    prep_chunk(0, 0)
                    prep_chunk(0, 1)
                for g in range(NG):
                    if b == 0 and ic == 0 and g + 2 < NG:
                        prep_chunk(0, g + 2)
                    if ic == NIC - 1 and b + 1 < B:
                        prep_chunk(b + 1, g)
                    group(b, ic, g, acc)
                for s in range(IC // 128):
                    rs = rs_pool.tile([128, 1], F32)
                    nc.vector.reciprocal(rs[:], acc[:, s * 256 + 128 : s * 256 + 129])
                    ob = ob_pool.tile([128, 128], F32)
                    nc.vector.tensor_scalar_mul(
                        ob[:], acc[:, s * 256 : s * 256 + 128], rs[:]
                    )
                    nc.sync.dma_start(
                        out_d[b, ic * IC + s * 128 : ic * IC + (s + 1) * 128, :],
                        ob[:],
                    )

    nc.compile()
    return nc


_CACHED_NC = None


def _get_program():
    global _CACHED_NC
    if _CACHED_NC is None:
        _CACHED_NC = build_program()
    return _CACHED_NC


def _split16(a):
    hi = a.astype(np.float16)
    lo = (a - hi.astype(np.float32)).astype(np.float16)
    return hi, lo


def make_in_maps(x, A_shape, q, k, v):
    x = np.ascontiguousarray(x, dtype=np.float32)
    xt = np.ascontiguousarray(x.transpose(0, 2, 1)).astype(np.float16)  # [B, D, N]
    q_hi, q_lo = _split16(np.ascontiguousarray(q, dtype=np.float32))
    k_hi, k_lo = _split16(np.ascontiguousarray(k, dtype=np.float32))
    v16 = np.ascontiguousarray(v, dtype=np.float32).astype(np.float16)
    in_maps = []
    for c in range(NCORES):
        r0 = c * RB
        xqt = np.ascontiguousarray(
            x[:, r0 : r0 + RB, :].transpose(0, 2, 1)
        ).astype(np.float16)
        maskT = np.ascontiguousarray(A_shape[r0 : r0 + RB, :].T, dtype=np.float16)
        in_maps.append(
            {
                "xt": xt,
                "xqt": xqt,
                "maskT": maskT,
                "q_hi": q_hi,
                "q_lo": q_lo,
                "k_hi": k_hi,
                "k_lo": k_lo,
                "v": v16,
            }
        )
    return in_maps


def kernel(x, A_shape, q, k, v):
    nc = _get_program()
    in_maps = make_in_maps(x, A_shape, q, k, v)
    res = run_bass_kernel_spmd(nc, in_maps, list(range(NCORES)))
    out = np.concatenate([res.results[c]["out"] for c in range(NCORES)], axis=1)
    return out.astype(np.float32)

